# revision 16
# baseline (speedup 1.0000x reference)
"""GATv2Stack Trainium2 kernel (8-core data-parallel over graphs).

bt=128 graphs of N=64 nodes, 16 graphs/core. See reference.py.
  h = x @ W_in + b_in
  2x: xl=h@Wl+bl; xr=h@Wr+br; e=att.lrelu(xr_i+xl_j); a=softmax_j(e+mask)
      g = a@(h@Wl+bl) + out_bias; g=ELU(g); g=LN(g); h=g+h; h=mask*h
  out = where(keep_graph, h, x@W_in+b_in)

Math restructurings:
  - lrelu identity: att.lrelu(xr_i+xl_j) = 0.8*att.max(xr_i,-xl_j)
    + (att.xl)_j + const_i (const_i cancels in softmax_j).
  - mask add is folded into the (att.xl)_j term (both j-broadcast).
  - softmax: exp(e-4) with accum_out sum -> z; alpha = exp(e-4-ln(z+eps)).
    All scalar-engine, no wide vector passes; et_w memset to -3e4 so
    unwritten lanes exp to 0 (z==0 rows get ln(eps), alpha==0).
  - ELU+LN: LN is shift-invariant so ELU's -1 drops; ELU+1 =
    min(exp(x),1)+relu(x) -> 2 scalar ops (fused +out_bias, PSUM read)
    + 1 vector scalar_tensor_tensor.
  - LN rstd = exp(-0.5*ln(var+eps)) keeps scalar engine in the one
    act table {exp,ln,relu,square,copy,identity}.
  - fp16 everywhere except residual h / LN stats (f32). fp16 matmul and
    transpose run 1 cycle/row on PE.

Per-core layouts (G=16 graphs, gp=g//2, par=g%2):
  hT[m]      [128,1024] f16  [m*128+p, g*64+v]
  h_node_w   [128,2048] f32  [par*64+v, gp*256+d]  (residual)
  xlT/xrT[m] [128,1024] f16  (with bias); xlTn = -xlT
  xl_nodes   [128,2048] f16  2 pair-orders (swap matmul)
  sl (g,hp)  [128,<=2048] f16 max(xr_i,-xl_j) per half (i0 chunk)
  e-mm: 4 streams (par,hp) tile_position -> psum rows 32s+t
  e_sb       [128,4096] f16  full (i*m+j) per gp; 8 scatter DMAs/gp
  et_w       [128,2048] f16  [par*64+i, gp*256+h*64+j]
  maddS_w    [128,512]  f32  [32s+t, gp*64+j] mask add (j-broadcast)
  mvec_w     [128,8] node mask
Scatter DMAs alternate sync/scalar queues (HWDGE); copies and LN
elementwise rotate vector/scalar/gpsimd per gp.
"""
import sys, os
sys.path.insert(0, '/opt/trn_rl_repo')
import numpy as np

import concourse.bass as bass
import concourse.mybir as mybir
from concourse import bass_utils, bacc
from concourse.tile import TileContext

dt = mybir.dt
F32, F16 = dt.float32, dt.float16
AF = mybir.ActivationFunctionType
ALU = mybir.AluOpType

B, T, N, D_IN = 2, 64, 64, 512
HID, L, H, C = 256, 2, 4, 64
BT = B * T
G = 16
NCORES = 8
NEG_SLOPE = 0.2
LN_EPS = 1e-5
NEG_BIG = -30000.0

_n = [0]
def _nm(p="t"):
    _n[0] += 1
    return f"{p}{_n[0]}"


def fd(ap, *dims):
    """Keep partition dim + offset of (sliced) AP, replace free dims."""
    return bass.AP(ap.tensor, ap.offset, [list(ap.ap[0])] + [[s, c] for (s, c) in dims])


def build_nc(mh=(64,) * G, ln_id=False, bz=False):
    nc = bacc.Bacc("TRN2", target_bir_lowering=False, debug=False,
                   enable_asserts=False, num_devices=1)

    def din(name, shape, dtype=F32):
        return nc.dram_tensor(name, list(shape), dtype, kind="ExternalInput").ap()

    x_d     = din("x_sh", [G * 64, D_IN], F16)
    win_d   = din("w_in", [D_IN, HID], F16)
    wl_d    = din("wl", [L, HID, HID], F16)
    wr_d    = din("wr", [L, HID, HID], F16)
    binT_d  = din("binT", [128, 2])
    blT_d   = din("blT", [128, 2 * L])
    brT_d   = din("brT", [128, 2 * L])
    obT_d   = din("obT", [128, 2 * L])
    att08_d = din("att08", [128, 2 * 2 * L], F16)
    att10_d = din("att10", [128, 2 * 2 * L], F16)
    gam_d   = din("gam_f", [L, 128, HID])
    bet_d   = din("bet_f", [L, 128, HID])
    swap_d  = din("swap16", [128, 128], F16)
    madds_d = din("maddS_w", [128, 8 * 64])
    mvec_d  = din("mvec_w", [128, 8])
    id16_d  = din("ident16", [128, 128], F16)
    id32_d  = din("ident32", [128, 128])
    out_d   = nc.dram_tensor("out", [G * 64, HID], F32, kind="ExternalOutput").ap()

    with TileContext(nc) as tc:
        with tc.tile_pool(name="const", bufs=1) as cpool, \
             tc.tile_pool(name="wide", bufs=1) as wpool, \
             tc.tile_pool(name="slp", bufs=2) as slpool, \
             tc.tile_pool(name="sm", bufs=2) as smpool, \
             tc.tile_pool(name="psum", bufs=1, space="PSUM") as ppool:

            ct_i = [0]
            def ctile(name, dram_ap, shape, dtype=F32):
                t0 = cpool.tile(shape, dtype, name=_nm(name))
                ct_i[0] += 1
                q = nc.sync if ct_i[0] % 2 == 0 else nc.scalar
                q.dma_start(t0[:], dram_ap)
                return t0

            # x rows + ident16 first so input transposes start while the
            # remaining consts stream in; alternate const loads across the
            # two DMA-capable queues.
            ident16 = ctile("id16", id16_d, [128, 128], F16)
            xrows = []
            x_rows_d = x_d.rearrange("(t p) d -> t p d", p=128)
            for t in range(8):
                xr_t = smpool.tile([128, D_IN], F16, name=_nm("xrow"), tag="xrow",
                                   bufs=8)
                (nc.sync if t % 2 == 0 else nc.scalar).dma_start(xr_t[:], x_rows_d[t])
                xrows.append(xr_t)
            win_r = win_d.rearrange("(k p) n -> k p n", p=128)
            win = [ctile(f"win{k}", win_r[k], [128, HID], F16) for k in range(4)]
            wl, wr = [], []
            for l in range(L):
                wl_r = wl_d[l].rearrange("(k p) n -> k p n", p=128)
                wr_r = wr_d[l].rearrange("(k p) n -> k p n", p=128)
                wl.append([ctile(f"wl{l}{k}", wl_r[k], [128, HID], F16) for k in range(2)])
                wr.append([ctile(f"wr{l}{k}", wr_r[k], [128, HID], F16) for k in range(2)])
            binT  = ctile("binT", binT_d, [128, 2])
            blT   = ctile("blT", blT_d, [128, 2 * L])
            brT   = ctile("brT", brT_d, [128, 2 * L])
            obT   = ctile("obT", obT_d, [128, 2 * L])
            att08 = ctile("att08", att08_d, [128, 2 * 2 * L], F16)
            att10 = ctile("att10", att10_d, [128, 2 * 2 * L], F16)
            gam   = [ctile(f"gam{l}", gam_d[l], [128, HID]) for l in range(L)]
            bet   = [ctile(f"bet{l}", bet_d[l], [128, HID]) for l in range(L)]
            swap16 = ctile("swap16", swap_d, [128, 128], F16)
            maddS = ctile("maddS", madds_d, [128, 8 * 64])
            mvec  = ctile("mvec", mvec_d, [128, 8])
            ident32 = ctile("id32", id32_d, [128, 128])
            c_n4 = cpool.tile([128, 1], F32, name=_nm("cn4"))
            nc.vector.memset(c_n4[:], -4.0)
            c_eps20 = cpool.tile([128, 1], F32, name=_nm("ceps20"))
            nc.vector.memset(c_eps20[:], 1e-20)
            c_lneps = cpool.tile([128, 1], F32, name=_nm("clneps"))
            nc.vector.memset(c_lneps[:], LN_EPS)
            c_zero = cpool.tile([128, 1], F32, name=_nm("czero"))
            nc.vector.memset(c_zero[:], 0.0)

            # rotate PSUM->SBUF copies between vector and scalar (gpsimd
            # cannot access PSUM)
            cp_i = [0]
            def cp(dst, src):
                cp_i[0] += 1
                if cp_i[0] % 2 == 0:
                    nc.scalar.copy(dst, src)
                else:
                    nc.vector.tensor_scalar(dst, src, 1.0, None, op0=ALU.mult)

            # ---------- input: load x (f16), transpose, project ----------
            hT = [smpool.tile([128, G * 64], F16, name=_nm("hT"), tag=f"hT{m}", bufs=2)
                  for m in range(2)]
            h_node_w = smpool.tile([128, 8 * HID], F32, name=_nm("hnode"), tag="hnode",
                                   bufs=2)

            with tc.tile_pool(name="xtp", bufs=1) as xtpool:
                xT = [xtpool.tile([128, G * 64], F16, name=_nm("xT")) for _ in range(4)]
                for t in range(8):
                    xrow = xrows[t]
                    for k in range(4):
                        pt = ppool.tile([128, 128], F16, name=_nm("pxt"), tag="tps16",
                                        bufs=2)
                        nc.tensor.transpose(pt[:], xrow[:, k * 128:(k + 1) * 128],
                                            ident16[:])
                        cp(xT[k][:, t * 128:(t + 1) * 128], pt[:])
                for m in range(2):
                    for cb in range(2):
                        ph = ppool.tile([128, 512], F32, name=_nm("ph"), tag="big",
                                        bufs=3)
                        for k in range(4):
                            nc.tensor.matmul(ph[:], win[k][:, m * 128:(m + 1) * 128],
                                             xT[k][:, cb * 512:(cb + 1) * 512],
                                             start=(k == 0), stop=(k == 3))
                        if cb == 0:
                            nc.vector.tensor_scalar(hT[m][:, cb * 512:(cb + 1) * 512],
                                                    ph[:], binT[:, m:m + 1], None,
                                                    op0=ALU.add)
                        else:
                            nc.scalar.activation(hT[m][:, cb * 512:(cb + 1) * 512],
                                                 ph[:], AF.Identity,
                                                 bias=binT[:, m:m + 1])
                for gp in range(8):
                    for m in range(2):
                        pt = ppool.tile([128, 128], F16, name=_nm("pnt"), tag="tps16",
                                        bufs=2)
                        nc.tensor.transpose(pt[:], hT[m][:, gp * 128:(gp + 1) * 128],
                                            ident16[:])
                        cp(h_node_w[:, gp * HID + m * 128:gp * HID + m * 128 + 128],
                           pt[:])

            # ---------- layers ----------
            for l in range(L):
                # residual + LN bias, precomputed wide (skipped when LN affine
                # is identity, the common case)
                if ln_id:
                    hb_w = h_node_w
                else:
                    hb_w = wpool.tile([128, 8 * HID], F32, name=_nm("hb"), tag="hb",
                                      bufs=2)
                    nc.vector.tensor_tensor(hb_w[:], h_node_w[:],
                                            fd(bet[l][0:128, 0:1], (0, 8), (1, HID)),
                                            op=ALU.add)
                et_w = wpool.tile([128, 8 * HID], F16, name=_nm("etw"), tag="etw",
                                  bufs=2)
                nc.gpsimd.memset(et_w[:], NEG_BIG)

                # xl/xr projections -> f16 with bias
                xlT = [smpool.tile([128, G * 64], F16, name=_nm("xlT"), tag=f"xlT{m}",
                                   bufs=2) for m in range(2)]
                xrT = [smpool.tile([128, G * 64], F16, name=_nm("xrT"), tag=f"xrT{m}",
                                   bufs=2) for m in range(2)]
                for wi, (W, Tt, bvec) in enumerate(((wl[l], xlT, blT), (wr[l], xrT, brT))):
                    for m in range(2):
                        for cb in range(2):
                            pp = ppool.tile([128, 512], F32, name=_nm("pp"), tag="big",
                                            bufs=3)
                            for k in range(2):
                                nc.tensor.matmul(pp[:], W[k][:, m * 128:(m + 1) * 128],
                                                 hT[k][:, cb * 512:(cb + 1) * 512],
                                                 start=(k == 0), stop=(k == 1))
                            if (wi * 4 + m * 2 + cb) % 2 == 0:
                                nc.scalar.activation(Tt[m][:, cb * 512:(cb + 1) * 512],
                                                     pp[:], AF.Identity,
                                                     bias=bvec[:, l * 2 + m:l * 2 + m + 1])
                            else:
                                nc.vector.tensor_scalar(Tt[m][:, cb * 512:(cb + 1) * 512],
                                                        pp[:],
                                                        bvec[:, l * 2 + m:l * 2 + m + 1],
                                                        None, op0=ALU.add)
                xlTn = [smpool.tile([128, G * 64], F16, name=_nm("xlTn"), tag=f"xlTn{m}",
                                    bufs=2) for m in range(2)]
                for m in range(2):
                    nc.vector.tensor_scalar(xlTn[m][:], xlT[m][:], -1.0, None,
                                            op0=ALU.mult)
                # xr duplicated pairwise: xr2[2v]=xr2[2v+1]=xr[v]. Gives the
                # sl broadcast ops packed inner dims (DVE 2x 16-bit mode).
                xr2 = [smpool.tile([128, 2 * G * 64], F16, name=_nm("xr2"),
                                   tag=f"xr2{m}", bufs=2) for m in range(2)]
                for m in range(2):
                    nc.scalar.copy(fd(xr2[m][0:128, 0:1], (2, G * 64), (1, 2)),
                                   fd(xrT[m][0:128, 0:1], (1, G * 64), (0, 2)))

                # xl in node layout (f16), two pair-orders
                xl_nodes = [smpool.tile([128, 8 * HID], F16, name=_nm("xlnode"),
                                        tag=f"xlnode{o}", bufs=2) for o in range(2)]
                for gp in range(8):
                    for m in range(2):
                        pt = ppool.tile([128, 128], F16, name=_nm("pxl"), tag="tps16",
                                        bufs=2)
                        nc.tensor.transpose(pt[:], xlT[m][:, gp * 128:(gp + 1) * 128],
                                            ident16[:])
                        cp(xl_nodes[0][:, gp * HID + m * 128:gp * HID + m * 128 + 128],
                           pt[:])
                for cb in range(4):
                    ps = ppool.tile([128, 512], F32, name=_nm("psw"), tag="big", bufs=3)
                    nc.tensor.matmul(ps[:], swap16[:],
                                     xl_nodes[0][:, cb * 512:(cb + 1) * 512],
                                     start=True, stop=True)
                    cp(xl_nodes[1][:, cb * 512:(cb + 1) * 512], ps[:])

                z4 = smpool.tile([128, 32], F32, name=_nm("z4"), tag="z4", bufs=2)
                b2 = smpool.tile([128, 32], F32, name=_nm("b2"), tag="b2", bufs=2)
                outT_w = wpool.tile([128, 2 * G * 64], F16, name=_nm("outT"),
                                    tag="outT", bufs=2)
                gn_w = wpool.tile([128, 8 * HID], F16, name=_nm("gnw"), tag="gnw",
                                  bufs=2)
                sums = smpool.tile([128, 8], F32, name=_nm("sums"), tag="sums", bufs=2)
                sqs = smpool.tile([128, 8], F32, name=_nm("sqs"), tag="sqs", bufs=2)
                mus = smpool.tile([128, 8], F32, name=_nm("mus"), tag="mus", bufs=2)
                vars_ = smpool.tile([128, 8], F32, name=_nm("vars"), tag="vars", bufs=2)
                lnv8 = smpool.tile([128, 8], F32, name=_nm("lnv8"), tag="lnv8", bufs=2)
                rstd8 = smpool.tile([128, 8], F32, name=_nm("rstd8"), tag="rstd8",
                                    bufs=2)
                hn_w = smpool.tile([128, 8 * HID], F32, name=_nm("hn"), tag="hnode",
                                   bufs=2)
                hT_nxt = None
                if l + 1 < L:
                    hT_nxt = [smpool.tile([128, G * 64], F16, name=_nm("hT"),
                                          tag=f"hT{m}", bufs=2) for m in range(2)]

                # ---- per graph-pair software pipeline ----
                def stage_a(gp):
                    m = mh[2 * gp]
                    # waxl = att.xl + mask (both broadcast over i)
                    paxl_t = ppool.tile([128, 256], F32, name=_nm("paxl"), tag="ops",
                                        bufs=2)
                    paxl = paxl_t[:, 0:64]
                    for par in range(2):
                        g = gp * 2 + par
                        for hp in range(2):
                            s_idx = par * 2 + hp
                            nc.tensor.matmul(
                                paxl_t[32 * s_idx:32 * s_idx + 2, 0:64],
                                att10[:, (l * 2 + hp) * 2:(l * 2 + hp) * 2 + 2],
                                xlT[hp][:, g * 64:(g + 1) * 64],
                                start=True, stop=True,
                                tile_position=(0, 32 * s_idx))
                    waxl = smpool.tile([128, 64], F16, name=_nm("waxl"), tag="waxl",
                                       bufs=3)
                    nc.vector.tensor_tensor(waxl[:], paxl,
                                            maddS[:, gp * 64:gp * 64 + 64], op=ALU.add)
                    e_sb = wpool.tile([128, 64 * 64], F16, name=_nm("esb"), tag="esb",
                                      bufs=2)
                    for half in range((m + 31) // 32):
                        i0 = half * 32
                        i_cnt = min(32, m - i0)
                        ipc = max(d for d in (1, 2, 4, 8, 16, 24, 32)
                                  if i_cnt % d == 0 and d * m <= 512)
                        nch = i_cnt // ipc
                        w = ipc * m
                        sls = {}
                        for par in range(2):
                            g = gp * 2 + par
                            for hp in range(2):
                                sl = slpool.tile([128, i_cnt * m], F16, name=_nm("sl"),
                                                 tag="sl", bufs=6,
                                                 padded_shape=[128, 32 * 64])
                                xr_sl = xr2[hp][:, (g * 64 + i0) * 2:
                                                (g * 64 + i0) * 2 + 1]
                                xl_sl = xlTn[hp][:, g * 64:g * 64 + 1]
                                nc.vector.tensor_tensor(
                                    fd(sl[0:128, 0:1], (m, i_cnt), (2, m // 2), (1, 2)),
                                    fd(xr_sl, (2, i_cnt), (0, m // 2), (1, 2)),
                                    fd(xl_sl, (0, i_cnt), (2, m // 2), (1, 2)),
                                    op=ALU.max)
                                sls[(par, hp)] = sl
                        for ci in range(nch):
                            pe = ppool.tile([128, 512], F32, name=_nm("pe"), tag="big",
                                            bufs=3)
                            for par in range(2):
                                for hp in range(2):
                                    s_idx = par * 2 + hp
                                    nc.tensor.matmul(
                                        pe[32 * s_idx:32 * s_idx + 2, 0:w],
                                        att08[:, (l * 2 + hp) * 2:(l * 2 + hp) * 2 + 2],
                                        sls[(par, hp)][:, ci * w:(ci + 1) * w],
                                        start=True, stop=True,
                                        tile_position=(0, 32 * s_idx))
                            dst_c = e_sb[:, i0 * m + ci * w:i0 * m + (ci + 1) * w]
                            if ci % 2 == 1 or nch == 1:
                                # vector path folds (att.xl + mask)_j for free
                                nc.vector.scalar_tensor_tensor(
                                    dst_c, pe[:, 0:w], 1.0,
                                    fd(waxl[0:128, 0:1], (0, ipc), (1, m)),
                                    op0=ALU.mult, op1=ALU.add)
                            else:
                                nc.scalar.copy(dst_c, pe[:, 0:w])
                        if nch > 1:
                            nce = (nch + 1) // 2       # scalar-copied chunks
                            base = e_sb[0:128, i0 * m:i0 * m + 1]
                            nc.vector.tensor_tensor(
                                fd(base, (2 * w, nce), (m, ipc), (1, m)),
                                fd(base, (2 * w, nce), (m, ipc), (1, m)),
                                fd(waxl[0:128, 0:1], (0, nce), (0, ipc), (1, m)),
                                op=ALU.add)
                    # scatter: stream rows -> [par*64+i, gp*HID+h*64+j]
                    for par in range(2):
                        for hp in range(2):
                            s_idx = par * 2 + hp
                            for t in range(2):
                                h_g = 2 * hp + t
                                src2 = fd(e_sb[32 * s_idx + t:32 * s_idx + t + 1, 0:1],
                                          (m, m), (1, m))
                                dst_base = et_w[par * 64:par * 64 + m,
                                                gp * HID + h_g * 64:
                                                gp * HID + h_g * 64 + 1]
                                dst = fd(dst_base, (1, m))
                                q = nc.scalar if (s_idx * 2 + t) in (1, 5) else nc.sync
                                q.dma_start(dst, src2)

                def stage_a2(gp):
                    # softmax: exp per head with accumulated z, then one
                    # alpha = exp * (1/z) broadcast multiply
                    scr = smpool.tile([128, HID], F32, name=_nm("scr"), tag="scr",
                                      bufs=3)
                    for h in range(4):
                        nc.scalar.activation(
                            scr[:, h * 64:h * 64 + 64],
                            et_w[:, gp * HID + h * 64:gp * HID + h * 64 + 64],
                            AF.Exp, bias=c_n4[:],
                            accum_out=z4[:, gp * 4 + h:gp * 4 + h + 1])
                    nc.vector.tensor_scalar(z4[:, gp * 4:gp * 4 + 4],
                                            z4[:, gp * 4:gp * 4 + 4], 1.0, 1e-20,
                                            op0=ALU.mult, op1=ALU.add)
                    for h in range(4):
                        nc.gpsimd.normalize_recip(
                            et_w[:, gp * HID + h * 64:gp * HID + h * 64 + 64],
                            scr[:, h * 64:h * 64 + 64],
                            z4[:, gp * 4 + h:gp * 4 + h + 1])

                def stage_b1(gp):
                    # alpha^T + out matmul + ELU(+1) into outT_w
                    po2 = ppool.tile([128, 256], F32, name=_nm("po"), tag="ops",
                                     bufs=2)
                    for hp in range(2):
                        pat = ppool.tile([128, 128], F16, name=_nm("pat"), tag="tps16",
                                         bufs=2)
                        nc.tensor.transpose(
                            pat[:], et_w[:, gp * HID + hp * 128:gp * HID + hp * 128 + 128],
                            ident16[:])
                        aT2 = smpool.tile([128, 128], F16, name=_nm("aT"), tag="aT",
                                          bufs=4)
                        cp(aT2[:], pat[:])
                        for par in range(2):
                            for t in range(2):
                                h_g = 2 * hp + t
                                xn = xl_nodes[0] if par == t else xl_nodes[1]
                                nc.tensor.matmul(
                                    po2[t * 64:(t + 1) * 64,
                                        hp * 128 + par * 64:hp * 128 + par * 64 + 64],
                                    xn[t * 64:t * 64 + 64,
                                       gp * HID + h_g * 64:gp * HID + h_g * 64 + 64],
                                    aT2[t * 64:t * 64 + 64, par * 64:par * 64 + 64],
                                    start=True, stop=True)
                    if bz:
                        # out_bias == 0: one ELU over both head-halves
                        e1 = smpool.tile([128, 256], F16, name=_nm("e1"), tag="e1",
                                         bufs=4)
                        nc.scalar.activation(e1[:], po2[:], AF.Exp, bias=c_zero[:])
                        r1 = smpool.tile([128, 256], F16, name=_nm("r1"), tag="r1",
                                         bufs=4)
                        nc.vector.tensor_scalar(r1[:], po2[:], 0.0, None, op0=ALU.max)
                        nc.vector.scalar_tensor_tensor(
                            outT_w[:, gp * 256:gp * 256 + 256], e1[:], 1.0, r1[:],
                            op0=ALU.min, op1=ALU.add)
                    else:
                        for hp in range(2):
                            po = po2[:, hp * 128:hp * 128 + 128]
                            ob_sl = obT[:, l * 2 + hp:l * 2 + hp + 1]
                            e1 = smpool.tile([128, 256], F16, name=_nm("e1"), tag="e1",
                                             bufs=4)
                            nc.scalar.activation(e1[:, 0:128], po, AF.Exp, bias=ob_sl)
                            r1 = smpool.tile([128, 256], F16, name=_nm("r1"), tag="r1",
                                             bufs=4)
                            nc.vector.tensor_scalar(r1[:, 0:128], po, ob_sl, 0.0,
                                                    op0=ALU.add, op1=ALU.max)
                            nc.vector.scalar_tensor_tensor(
                                outT_w[:, gp * 256 + hp * 128:gp * 256 + hp * 128 + 128],
                                e1[:, 0:128], 1.0, r1[:, 0:128],
                                op0=ALU.min, op1=ALU.add)
                    # node layout + LayerNorm + residual + mask, per gp
                    for hp in range(2):
                        pg = ppool.tile([128, 128], F16, name=_nm("pg"), tag="tps16",
                                        bufs=2)
                        nc.tensor.transpose(
                            pg[:], outT_w[:, gp * 256 + hp * 128:gp * 256 + hp * 128 + 128],
                            ident16[:])
                        cp(gn_w[:, gp * HID + hp * 128:gp * HID + hp * 128 + 128], pg[:])
                    gsl = gn_w[:, gp * HID:gp * HID + HID]
                    nc.vector.tensor_reduce(sums[:, gp:gp + 1], gsl,
                                            axis=mybir.AxisListType.X, op=ALU.add)
                    scr2 = smpool.tile([128, HID], F16, name=_nm("scr2"), tag="scr2",
                                       bufs=2)
                    nc.scalar.activation(scr2[:], gsl, AF.Square, bias=c_zero[:],
                                         accum_out=sqs[:, gp:gp + 1])
                    nc.vector.tensor_scalar(mus[:, gp:gp + 1], sums[:, gp:gp + 1],
                                            1.0 / HID, None, op0=ALU.mult)
                    nc.vector.tensor_tensor(vars_[:, gp:gp + 1], mus[:, gp:gp + 1],
                                            mus[:, gp:gp + 1], op=ALU.mult)
                    nc.vector.scalar_tensor_tensor(vars_[:, gp:gp + 1],
                                                   sqs[:, gp:gp + 1], 1.0 / HID,
                                                   vars_[:, gp:gp + 1],
                                                   op0=ALU.mult, op1=ALU.subtract)

                def rstd_batch(b):
                    # batched so the scalar engine flips act tables only twice
                    # per layer (ln then back to exp)
                    sl4 = slice(b * 4, b * 4 + 4)
                    nc.scalar.activation(lnv8[:, sl4], vars_[:, sl4], AF.Ln,
                                         bias=c_lneps[:])
                    nc.scalar.activation(rstd8[:, sl4], lnv8[:, sl4], AF.Exp,
                                         bias=c_zero[:], scale=-0.5)

                def stage_b2(gp):
                    gsl = gn_w[:, gp * HID:gp * HID + HID]
                    nc.vector.tensor_scalar(gsl, gsl, mus[:, gp:gp + 1],
                                            rstd8[:, gp:gp + 1],
                                            op0=ALU.subtract, op1=ALU.mult)
                    if not ln_id:
                        nc.vector.tensor_tensor(gsl, gsl, gam[l][:], op=ALU.mult)
                    hsl = hn_w[:, gp * HID:gp * HID + HID]
                    nc.vector.tensor_tensor(hsl, gsl, hb_w[:, gp * HID:gp * HID + HID],
                                            op=ALU.add)
                    nc.vector.tensor_scalar(hsl, hsl, mvec[:, gp:gp + 1], None,
                                            op0=ALU.mult)
                    if hT_nxt is not None:
                        for m in range(2):
                            pt2 = ppool.tile([128, 128], F32, name=_nm("pht"),
                                             tag="tpsf", bufs=1)
                            nc.tensor.transpose(
                                pt2[:],
                                hn_w[:, gp * HID + m * 128:gp * HID + m * 128 + 128],
                                ident32[:])
                            cp(hT_nxt[m][:, gp * 128:(gp + 1) * 128], pt2[:])
                    else:
                        # last layer: stream this pair's rows out now
                        for par in range(2):
                            g = gp * 2 + par
                            src = fd(hn_w[par * 64:par * 64 + 64,
                                          gp * HID:gp * HID + 1], (1, HID))
                            dst_sl = out_d[g * 64:g * 64 + 1, :]
                            dst = bass.AP(dst_sl.tensor, dst_sl.offset,
                                          [[HID, 64], [1, HID]])
                            q = nc.sync if par == 0 else nc.scalar
                            q.dma_start(dst, src)

                # software pipeline: scatters of gp complete while gp+1
                # computes; softmax of gp runs while gp-1 finishes; the
                # normalize tail runs in two rstd batches.
                for gp in range(8):
                    stage_a(gp)
                    if gp >= 1:
                        stage_a2(gp - 1)
                    if gp >= 2:
                        stage_b1(gp - 2)
                    if gp == 5:
                        rstd_batch(0)
                    if gp >= 5:
                        stage_b2(gp - 5)
                stage_a2(7)
                stage_b1(6)
                stage_b2(3)
                stage_b1(7)
                rstd_batch(1)
                for gp in range(4, 8):
                    stage_b2(gp)

                h_node_w = hn_w
                if hT_nxt is not None:
                    hT = hT_nxt

    nc.finalize()
    return nc


_CACHE = {}

def _get_nc(mh, ln_id=False, bz=False):
    key = (tuple(mh), ln_id, bz)
    if key not in _CACHE:
        _CACHE[key] = build_nc(tuple(mh), ln_id, bz)
    return _CACHE[key]


def _host_prep(x, person_mask, W_in, b_in, Wl, bl, Wr, br, att, out_bias, ln_scale, ln_bias):
    x = np.asarray(x, np.float32).reshape(BT, N, D_IN)
    m = np.asarray(person_mask).reshape(BT, N)
    W_in = np.ascontiguousarray(np.asarray(W_in, np.float16))
    b_in = np.asarray(b_in, np.float32)
    Wl = np.ascontiguousarray(np.asarray(Wl, np.float16))
    bl = np.asarray(bl, np.float32)
    Wr = np.ascontiguousarray(np.asarray(Wr, np.float16))
    br = np.asarray(br, np.float32)
    att = np.asarray(att, np.float32)
    out_bias = np.asarray(out_bias, np.float32)
    ln_scale = np.asarray(ln_scale, np.float32)
    ln_bias = np.asarray(ln_bias, np.float32)

    # ---- pack active nodes; stripe sorted graphs across cores ----
    n_g = m.sum(-1).astype(np.int64)                     # active counts
    order = np.argsort(-n_g, kind="stable")              # desc
    idxs = [np.nonzero(m[g])[0] for g in range(BT)]
    mh = []
    for s in range(G):
        n_top = n_g[order[s * NCORES]]
        mh.append(max(8, int(-(-int(n_top) // 8) * 8)))
    for k in range(0, G, 2):                             # pair-equalize
        mh[k + 1] = mh[k]
    mh = tuple(min(64, v) for v in mh)

    binT = np.zeros((128, 2), np.float32)
    for mm in range(2):
        binT[:, mm] = b_in[mm * 128:(mm + 1) * 128]
    blT = np.zeros((128, 2 * L), np.float32)
    brT = np.zeros((128, 2 * L), np.float32)
    obT = np.zeros((128, 2 * L), np.float32)
    for l in range(L):
        for mm in range(2):
            blT[:, l * 2 + mm] = bl[l, mm * 128:(mm + 1) * 128]
            brT[:, l * 2 + mm] = br[l, mm * 128:(mm + 1) * 128]
            obT[:, l * 2 + mm] = out_bias[l, mm * 128:(mm + 1) * 128]
    attBD = np.zeros((128, 2 * 2 * L), np.float32)
    for l in range(L):
        for hp in range(2):
            for t in range(2):
                attBD[t * 64:(t + 1) * 64, (l * 2 + hp) * 2 + t] = att[l, 2 * hp + t]
    att08 = (attBD * 0.8).astype(np.float16)
    att10 = attBD.astype(np.float16)
    gam_f = np.repeat(ln_scale[:, None, :], 128, 1).astype(np.float32).copy()
    bet_f = np.repeat(ln_bias[:, None, :], 128, 1).astype(np.float32).copy()
    ident16 = np.eye(128, dtype=np.float16)
    ident32 = np.eye(128, dtype=np.float32)
    swap16 = np.zeros((128, 128), np.float16)
    swap16[0:64, 64:128] = np.eye(64)
    swap16[64:128, 0:64] = np.eye(64)

    in_maps = []
    for c in range(NCORES):
        xg = np.zeros((G * 64, D_IN), np.float16)
        madd = np.full((2, 8, 64), NEG_BIG, np.float32)   # [par, gp, j]
        mvec_w = np.zeros((128, 8), np.float32)
        for s in range(G):
            gg = order[s * NCORES + c]
            n = int(n_g[gg])
            gp, par = s // 2, s % 2
            if n > 0:
                xg[s * 64:s * 64 + n] = x[gg][idxs[gg]].astype(np.float16)
                madd[par, gp, 0:n] = 0.0
                mvec_w[par * 64 + np.arange(n), gp] = 1.0
            else:
                madd[par, gp, 0] = 0.0
        maddS = np.zeros((128, 8 * 64), np.float32)
        for par in range(2):
            for hp in range(2):
                for t in range(2):
                    row = 32 * (par * 2 + hp) + t
                    maddS[row] = madd[par].reshape(-1)
        in_maps.append({
            "x_sh": xg, "w_in": W_in, "wl": Wl, "wr": Wr,
            "binT": binT, "blT": blT, "brT": brT, "obT": obT,
            "att08": att08, "att10": att10, "gam_f": gam_f, "bet_f": bet_f,
            "swap16": swap16, "maddS_w": maddS, "mvec_w": mvec_w,
            "ident16": ident16, "ident32": ident32,
        })
    return in_maps, x, m, W_in, b_in, order, idxs, n_g, mh


def kernel(**inputs) -> np.ndarray:
    in_maps, x, m, W_in, b_in, order, idxs, n_g, mh = _host_prep(**inputs)
    ln_id = (np.all(np.asarray(inputs["ln_scale"]) == 1.0)
             and np.all(np.asarray(inputs["ln_bias"]) == 0.0))
    bz = np.all(np.asarray(inputs["out_bias"]) == 0.0)
    nc = _get_nc(mh, ln_id, bz)
    res = bass_utils.run_bass_kernel_spmd(nc, in_maps, core_ids=list(range(NCORES)))
    out = np.zeros((BT, N, HID), np.float32)
    for c in range(NCORES):
        dev = res.results[c]["out"].reshape(G, 64, HID)
        for s in range(G):
            gg = order[s * NCORES + c]
            n = int(n_g[gg])
            if n > 0:
                out[gg][idxs[gg]] = dev[s, :n]
    keep = n_g > 1
    if not keep.all():
        for g in np.nonzero(~keep)[0]:
            out[g] = (x[g].astype(np.float32) @ np.asarray(W_in, np.float32)
                      + b_in)
    return out.reshape(B, T, N, HID)


# revision 18
# speedup vs baseline: 1.0842x; 1.0842x over previous
"""GATv2Stack Trainium2 kernel (8-core data-parallel over graphs).

bt=128 graphs of N=64 nodes, 16 graphs/core. See reference.py.
  h = x @ W_in + b_in
  2x: xl=h@Wl+bl; xr=h@Wr+br; e=att.lrelu(xr_i+xl_j); a=softmax_j(e+mask)
      g = a@(h@Wl+bl) + out_bias; g=ELU(g); g=LN(g); h=g+h; h=mask*h
  out = where(keep_graph, h, x@W_in+b_in)

Math restructurings:
  - lrelu identity: att.lrelu(xr_i+xl_j) = 0.8*att.max(xr_i,-xl_j)
    + (att.xl)_j + const_i (const_i cancels in softmax_j).
  - mask add is folded into the (att.xl)_j term (both j-broadcast).
  - softmax: exp(e-4) with accum_out sum -> z; alpha = exp(e-4-ln(z+eps)).
    All scalar-engine, no wide vector passes; et_w memset to -3e4 so
    unwritten lanes exp to 0 (z==0 rows get ln(eps), alpha==0).
  - ELU+LN: LN is shift-invariant so ELU's -1 drops; ELU+1 =
    min(exp(x),1)+relu(x) -> 2 scalar ops (fused +out_bias, PSUM read)
    + 1 vector scalar_tensor_tensor.
  - LN rstd = exp(-0.5*ln(var+eps)) keeps scalar engine in the one
    act table {exp,ln,relu,square,copy,identity}.
  - fp16 everywhere except residual h / LN stats (f32). fp16 matmul and
    transpose run 1 cycle/row on PE.

Per-core layouts (G=16 graphs, gp=g//2, par=g%2):
  hT[m]      [128,1024] f16  [m*128+p, g*64+v]
  h_node_w   [128,2048] f32  [par*64+v, gp*256+d]  (residual)
  xlT/xrT[m] [128,1024] f16  (with bias); xlTn = -xlT
  xl_nodes   [128,2048] f16  2 pair-orders (swap matmul)
  sl (g,hp)  [128,<=2048] f16 max(xr_i,-xl_j) per half (i0 chunk)
  e-mm: 4 streams (par,hp) tile_position -> psum rows 32s+t
  e_sb       [128,4096] f16  full (i*m+j) per gp; 8 scatter DMAs/gp
  et_w       [128,2048] f16  [par*64+i, gp*256+h*64+j]
  maddS_w    [128,512]  f32  [32s+t, gp*64+j] mask add (j-broadcast)
  mvec_w     [128,8] node mask
Scatter DMAs alternate sync/scalar queues (HWDGE); copies and LN
elementwise rotate vector/scalar/gpsimd per gp.
"""
import sys, os
sys.path.insert(0, '/opt/trn_rl_repo')
import numpy as np

import concourse.bass as bass
import concourse.mybir as mybir
from concourse import bass_utils, bacc
from concourse.tile import TileContext

dt = mybir.dt
F32, F16 = dt.float32, dt.float16
AF = mybir.ActivationFunctionType
ALU = mybir.AluOpType

B, T, N, D_IN = 2, 64, 64, 512
HID, L, H, C = 256, 2, 4, 64
BT = B * T
G = 16
NCORES = 8
NEG_SLOPE = 0.2
LN_EPS = 1e-5
NEG_BIG = -30000.0

_n = [0]
def _nm(p="t"):
    _n[0] += 1
    return f"{p}{_n[0]}"


def fd(ap, *dims):
    """Keep partition dim + offset of (sliced) AP, replace free dims."""
    return bass.AP(ap.tensor, ap.offset, [list(ap.ap[0])] + [[s, c] for (s, c) in dims])


def build_nc(mh=(64,) * G, ln_id=False, bz=False):
    nc = bacc.Bacc("TRN2", target_bir_lowering=False, debug=False,
                   enable_asserts=False, num_devices=1)

    def din(name, shape, dtype=F32):
        return nc.dram_tensor(name, list(shape), dtype, kind="ExternalInput").ap()

    x_d     = din("x_sh", [G * 64, D_IN], F16)
    win_d   = din("w_in", [D_IN, HID], F16)
    wl_d    = din("wl", [L, HID, HID], F16)
    wr_d    = din("wr", [L, HID, HID], F16)
    binT_d  = din("binT", [128, 2])
    blT_d   = din("blT", [128, 2 * L])
    brT_d   = din("brT", [128, 2 * L])
    obT_d   = din("obT", [128, 2 * L])
    att08_d = din("att08", [128, 2 * 2 * L], F16)
    att10_d = din("att10", [128, 2 * 2 * L], F16)
    gam_d   = din("gam_f", [L, 128, HID])
    bet_d   = din("bet_f", [L, 128, HID])
    swap_d  = din("swap16", [128, 128], F16)
    madds_d = din("maddS_w", [128, 8 * 64])
    mvec_d  = din("mvec_w", [128, 8])
    id16_d  = din("ident16", [128, 128], F16)
    id32_d  = din("ident32", [128, 128])
    out_d   = nc.dram_tensor("out", [G * 64, HID], F32, kind="ExternalOutput").ap()

    with TileContext(nc) as tc:
        with tc.tile_pool(name="const", bufs=1) as cpool, \
             tc.tile_pool(name="wide", bufs=1) as wpool, \
             tc.tile_pool(name="slp", bufs=2) as slpool, \
             tc.tile_pool(name="sm", bufs=2) as smpool, \
             tc.tile_pool(name="psum", bufs=1, space="PSUM") as ppool:

            def ctile(name, dram_ap, shape, dtype=F32):
                t0 = cpool.tile(shape, dtype, name=_nm(name))
                nc.sync.dma_start(t0[:], dram_ap)
                return t0

            # x rows + ident16 first so input transposes start while the
            # remaining consts stream in; alternate const loads across the
            # two DMA-capable queues.
            ident16 = ctile("id16", id16_d, [128, 128], F16)
            xrows = []
            x_rows_d = x_d.rearrange("(t p) d -> t p d", p=128)
            for t in range(8):
                xr_t = smpool.tile([128, D_IN], F16, name=_nm("xrow"), tag="xrow",
                                   bufs=8)
                (nc.sync if t % 2 == 0 else nc.scalar).dma_start(xr_t[:], x_rows_d[t])
                xrows.append(xr_t)
            win_r = win_d.rearrange("(k p) n -> k p n", p=128)
            win = [ctile(f"win{k}", win_r[k], [128, HID], F16) for k in range(4)]
            wl, wr = [], []
            for l in range(L):
                wl_r = wl_d[l].rearrange("(k p) n -> k p n", p=128)
                wr_r = wr_d[l].rearrange("(k p) n -> k p n", p=128)
                wl.append([ctile(f"wl{l}{k}", wl_r[k], [128, HID], F16) for k in range(2)])
                wr.append([ctile(f"wr{l}{k}", wr_r[k], [128, HID], F16) for k in range(2)])
            binT  = ctile("binT", binT_d, [128, 2])
            blT   = ctile("blT", blT_d, [128, 2 * L])
            brT   = ctile("brT", brT_d, [128, 2 * L])
            obT   = ctile("obT", obT_d, [128, 2 * L])
            att08 = ctile("att08", att08_d, [128, 2 * 2 * L], F16)
            att10 = ctile("att10", att10_d, [128, 2 * 2 * L], F16)
            gam   = [ctile(f"gam{l}", gam_d[l], [128, HID]) for l in range(L)]
            bet   = [ctile(f"bet{l}", bet_d[l], [128, HID]) for l in range(L)]
            swap16 = ctile("swap16", swap_d, [128, 128], F16)
            maddS = ctile("maddS", madds_d, [128, 8 * 64])
            mvec  = ctile("mvec", mvec_d, [128, 8])
            ident32 = ctile("id32", id32_d, [128, 128])
            c_n4 = cpool.tile([128, 1], F32, name=_nm("cn4"))
            nc.vector.memset(c_n4[:], -4.0)
            c_eps20 = cpool.tile([128, 1], F32, name=_nm("ceps20"))
            nc.vector.memset(c_eps20[:], 1e-20)
            c_lneps = cpool.tile([128, 1], F32, name=_nm("clneps"))
            nc.vector.memset(c_lneps[:], LN_EPS)
            c_zero = cpool.tile([128, 1], F32, name=_nm("czero"))
            nc.vector.memset(c_zero[:], 0.0)

            # rotate PSUM->SBUF copies between vector and scalar (gpsimd
            # cannot access PSUM)
            cp_i = [0]
            def cp(dst, src):
                cp_i[0] += 1
                if cp_i[0] % 2 == 0:
                    nc.scalar.copy(dst, src)
                else:
                    nc.vector.tensor_scalar(dst, src, 1.0, None, op0=ALU.mult)

            # ---------- input: load x (f16), transpose, project ----------
            hT = [smpool.tile([128, G * 64], F16, name=_nm("hT"), tag=f"hT{m}", bufs=2)
                  for m in range(2)]
            h_node_w = smpool.tile([128, 8 * HID], F32, name=_nm("hnode"), tag="hnode",
                                   bufs=2)

            with tc.tile_pool(name="xtp", bufs=1) as xtpool:
                xT = [xtpool.tile([128, G * 64], F16, name=_nm("xT")) for _ in range(4)]
                for t in range(8):
                    xrow = xrows[t]
                    for k in range(4):
                        pt = ppool.tile([128, 128], F16, name=_nm("pxt"), tag="tps16",
                                        bufs=2)
                        nc.tensor.transpose(pt[:], xrow[:, k * 128:(k + 1) * 128],
                                            ident16[:])
                        nc.vector.tensor_scalar(xT[k][:, t * 128:(t + 1) * 128],
                                                pt[:], 1.0, None, op0=ALU.mult)
                for m in range(2):
                    for cb in range(2):
                        ph = ppool.tile([128, 512], F32, name=_nm("ph"), tag="big",
                                        bufs=3)
                        for k in range(4):
                            nc.tensor.matmul(ph[:], win[k][:, m * 128:(m + 1) * 128],
                                             xT[k][:, cb * 512:(cb + 1) * 512],
                                             start=(k == 0), stop=(k == 3))
                        if cb == 0:
                            nc.vector.tensor_scalar(hT[m][:, cb * 512:(cb + 1) * 512],
                                                    ph[:], binT[:, m:m + 1], None,
                                                    op0=ALU.add)
                        else:
                            nc.scalar.activation(hT[m][:, cb * 512:(cb + 1) * 512],
                                                 ph[:], AF.Identity,
                                                 bias=binT[:, m:m + 1])
                for gp in range(8):
                    for m in range(2):
                        pt = ppool.tile([128, 128], F16, name=_nm("pnt"), tag="tps16",
                                        bufs=2)
                        nc.tensor.transpose(pt[:], hT[m][:, gp * 128:(gp + 1) * 128],
                                            ident16[:])
                        cp(h_node_w[:, gp * HID + m * 128:gp * HID + m * 128 + 128],
                           pt[:])

            # ---------- layers ----------
            for l in range(L):
                # residual + LN bias, precomputed wide (skipped when LN affine
                # is identity, the common case)
                if ln_id:
                    hb_w = h_node_w
                else:
                    hb_w = wpool.tile([128, 8 * HID], F32, name=_nm("hb"), tag="hb",
                                      bufs=2)
                    nc.vector.tensor_tensor(hb_w[:], h_node_w[:],
                                            fd(bet[l][0:128, 0:1], (0, 8), (1, HID)),
                                            op=ALU.add)
                et_w = wpool.tile([128, 8 * HID], F16, name=_nm("etw"), tag="etw",
                                  bufs=2)
                nc.gpsimd.memset(et_w[:], NEG_BIG)

                # xl/xr projections -> f16 with bias
                xlT = [smpool.tile([128, G * 64], F16, name=_nm("xlT"), tag=f"xlT{m}",
                                   bufs=2) for m in range(2)]
                xrT = [smpool.tile([128, G * 64], F16, name=_nm("xrT"), tag=f"xrT{m}",
                                   bufs=2) for m in range(2)]
                for wi, (W, Tt, bvec) in enumerate(((wl[l], xlT, blT), (wr[l], xrT, brT))):
                    for m in range(2):
                        for cb in range(2):
                            pp = ppool.tile([128, 512], F32, name=_nm("pp"), tag="big",
                                            bufs=3)
                            for k in range(2):
                                nc.tensor.matmul(pp[:], W[k][:, m * 128:(m + 1) * 128],
                                                 hT[k][:, cb * 512:(cb + 1) * 512],
                                                 start=(k == 0), stop=(k == 1))
                            if (wi * 4 + m * 2 + cb) % 2 == 0:
                                nc.scalar.activation(Tt[m][:, cb * 512:(cb + 1) * 512],
                                                     pp[:], AF.Identity,
                                                     bias=bvec[:, l * 2 + m:l * 2 + m + 1])
                            else:
                                nc.vector.tensor_scalar(Tt[m][:, cb * 512:(cb + 1) * 512],
                                                        pp[:],
                                                        bvec[:, l * 2 + m:l * 2 + m + 1],
                                                        None, op0=ALU.add)
                xlTn = [smpool.tile([128, G * 64], F16, name=_nm("xlTn"), tag=f"xlTn{m}",
                                    bufs=2) for m in range(2)]
                for m in range(2):
                    nc.vector.tensor_scalar(xlTn[m][:], xlT[m][:], -1.0, None,
                                            op0=ALU.mult)
                # xr duplicated pairwise: xr2[2v]=xr2[2v+1]=xr[v]. Gives the
                # sl broadcast ops packed inner dims (DVE 2x 16-bit mode).
                xr2 = [smpool.tile([128, 2 * G * 64], F16, name=_nm("xr2"),
                                   tag=f"xr2{m}", bufs=2) for m in range(2)]
                for m in range(2):
                    nc.scalar.copy(fd(xr2[m][0:128, 0:1], (2, G * 64), (1, 2)),
                                   fd(xrT[m][0:128, 0:1], (1, G * 64), (0, 2)))

                # xl in node layout (f16), two pair-orders
                xl_nodes = [smpool.tile([128, 8 * HID], F16, name=_nm("xlnode"),
                                        tag=f"xlnode{o}", bufs=2) for o in range(2)]
                for gp in range(8):
                    for m in range(2):
                        pt = ppool.tile([128, 128], F16, name=_nm("pxl"), tag="tps16",
                                        bufs=2)
                        nc.tensor.transpose(pt[:], xlT[m][:, gp * 128:(gp + 1) * 128],
                                            ident16[:])
                        cp(xl_nodes[0][:, gp * HID + m * 128:gp * HID + m * 128 + 128],
                           pt[:])
                for cb in range(4):
                    ps = ppool.tile([128, 512], F32, name=_nm("psw"), tag="big", bufs=3)
                    nc.tensor.matmul(ps[:], swap16[:],
                                     xl_nodes[0][:, cb * 512:(cb + 1) * 512],
                                     start=True, stop=True)
                    cp(xl_nodes[1][:, cb * 512:(cb + 1) * 512], ps[:])

                z4 = smpool.tile([128, 32], F32, name=_nm("z4"), tag="z4", bufs=2)
                b2 = smpool.tile([128, 32], F32, name=_nm("b2"), tag="b2", bufs=2)
                outT_w = wpool.tile([128, 2 * G * 64], F16, name=_nm("outT"),
                                    tag="outT", bufs=2)
                gn_w = wpool.tile([128, 8 * HID], F16, name=_nm("gnw"), tag="gnw",
                                  bufs=2)
                sums = smpool.tile([128, 8], F32, name=_nm("sums"), tag="sums", bufs=2)
                sqs = smpool.tile([128, 8], F32, name=_nm("sqs"), tag="sqs", bufs=2)
                mus = smpool.tile([128, 8], F32, name=_nm("mus"), tag="mus", bufs=2)
                vars_ = smpool.tile([128, 8], F32, name=_nm("vars"), tag="vars", bufs=2)
                lnv8 = smpool.tile([128, 8], F32, name=_nm("lnv8"), tag="lnv8", bufs=2)
                rstd8 = smpool.tile([128, 8], F32, name=_nm("rstd8"), tag="rstd8",
                                    bufs=2)
                hn_w = smpool.tile([128, 8 * HID], F32, name=_nm("hn"), tag="hnode",
                                   bufs=2)
                hT_nxt = None
                if l + 1 < L:
                    hT_nxt = [smpool.tile([128, G * 64], F16, name=_nm("hT"),
                                          tag=f"hT{m}", bufs=2) for m in range(2)]

                # ---- per graph-pair software pipeline ----
                def stage_a(gp):
                    m = mh[2 * gp]
                    # waxl = att.xl + mask (both broadcast over i)
                    paxl_t = ppool.tile([128, 256], F32, name=_nm("paxl"), tag="ops",
                                        bufs=2)
                    paxl = paxl_t[:, 0:64]
                    for par in range(2):
                        g = gp * 2 + par
                        for hp in range(2):
                            s_idx = par * 2 + hp
                            nc.tensor.matmul(
                                paxl_t[32 * s_idx:32 * s_idx + 2, 0:64],
                                att10[:, (l * 2 + hp) * 2:(l * 2 + hp) * 2 + 2],
                                xlT[hp][:, g * 64:(g + 1) * 64],
                                start=True, stop=True,
                                tile_position=(0, 32 * s_idx))
                    waxl = smpool.tile([128, 64], F16, name=_nm("waxl"), tag="waxl",
                                       bufs=3)
                    nc.vector.tensor_tensor(waxl[:], paxl,
                                            maddS[:, gp * 64:gp * 64 + 64], op=ALU.add)
                    e_sb = wpool.tile([128, 64 * 64], F16, name=_nm("esb"), tag="esb",
                                      bufs=2)
                    for half in range((m + 31) // 32):
                        i0 = half * 32
                        i_cnt = min(32, m - i0)
                        ipc = max(d for d in (1, 2, 4, 8, 16, 24, 32)
                                  if i_cnt % d == 0 and d * m <= 512)
                        nch = i_cnt // ipc
                        w = ipc * m
                        sls = {}
                        for par in range(2):
                            g = gp * 2 + par
                            for hp in range(2):
                                sl = slpool.tile([128, i_cnt * m], F16, name=_nm("sl"),
                                                 tag="sl", bufs=6,
                                                 padded_shape=[128, 32 * 64])
                                xr_sl = xr2[hp][:, (g * 64 + i0) * 2:
                                                (g * 64 + i0) * 2 + 1]
                                xl_sl = xlTn[hp][:, g * 64:g * 64 + 1]
                                nc.vector.tensor_tensor(
                                    fd(sl[0:128, 0:1], (m, i_cnt), (2, m // 2), (1, 2)),
                                    fd(xr_sl, (2, i_cnt), (0, m // 2), (1, 2)),
                                    fd(xl_sl, (0, i_cnt), (2, m // 2), (1, 2)),
                                    op=ALU.max)
                                sls[(par, hp)] = sl
                        for ci in range(nch):
                            pe = ppool.tile([128, 512], F32, name=_nm("pe"), tag="big",
                                            bufs=3)
                            for par in range(2):
                                for hp in range(2):
                                    s_idx = par * 2 + hp
                                    nc.tensor.matmul(
                                        pe[32 * s_idx:32 * s_idx + 2, 0:w],
                                        att08[:, (l * 2 + hp) * 2:(l * 2 + hp) * 2 + 2],
                                        sls[(par, hp)][:, ci * w:(ci + 1) * w],
                                        start=True, stop=True,
                                        tile_position=(0, 32 * s_idx))
                            dst_c = e_sb[:, i0 * m + ci * w:i0 * m + (ci + 1) * w]
                            if ci % 2 == 1 or nch == 1:
                                # vector path folds (att.xl + mask)_j for free
                                nc.vector.scalar_tensor_tensor(
                                    dst_c, pe[:, 0:w], 1.0,
                                    fd(waxl[0:128, 0:1], (0, ipc), (1, m)),
                                    op0=ALU.mult, op1=ALU.add)
                            else:
                                nc.scalar.copy(dst_c, pe[:, 0:w])
                        if nch > 1:
                            nce = (nch + 1) // 2       # scalar-copied chunks
                            base = e_sb[0:128, i0 * m:i0 * m + 1]
                            nc.vector.tensor_tensor(
                                fd(base, (2 * w, nce), (m, ipc), (1, m)),
                                fd(base, (2 * w, nce), (m, ipc), (1, m)),
                                fd(waxl[0:128, 0:1], (0, nce), (0, ipc), (1, m)),
                                op=ALU.add)
                    # scatter: stream rows -> [par*64+i, gp*HID+h*64+j]
                    for par in range(2):
                        for hp in range(2):
                            s_idx = par * 2 + hp
                            for t in range(2):
                                h_g = 2 * hp + t
                                src2 = fd(e_sb[32 * s_idx + t:32 * s_idx + t + 1, 0:1],
                                          (m, m), (1, m))
                                dst_base = et_w[par * 64:par * 64 + m,
                                                gp * HID + h_g * 64:
                                                gp * HID + h_g * 64 + 1]
                                dst = fd(dst_base, (1, m))
                                q = nc.scalar if (s_idx * 2 + t) in (1, 5) else nc.sync
                                q.dma_start(dst, src2)

                def stage_a2(gp):
                    # softmax: exp per head with accumulated z, then one
                    # alpha = exp * (1/z) broadcast multiply
                    scr = smpool.tile([128, HID], F32, name=_nm("scr"), tag="scr",
                                      bufs=3)
                    for h in range(4):
                        nc.scalar.activation(
                            scr[:, h * 64:h * 64 + 64],
                            et_w[:, gp * HID + h * 64:gp * HID + h * 64 + 64],
                            AF.Exp, bias=c_n4[:],
                            accum_out=z4[:, gp * 4 + h:gp * 4 + h + 1])
                    nc.vector.tensor_scalar(z4[:, gp * 4:gp * 4 + 4],
                                            z4[:, gp * 4:gp * 4 + 4], 1.0, 1e-20,
                                            op0=ALU.mult, op1=ALU.add)
                    for h in range(4):
                        nc.gpsimd.normalize_recip(
                            et_w[:, gp * HID + h * 64:gp * HID + h * 64 + 64],
                            scr[:, h * 64:h * 64 + 64],
                            z4[:, gp * 4 + h:gp * 4 + h + 1])

                def stage_b1(gp):
                    # alpha^T + out matmul + ELU(+1) into outT_w
                    po2 = ppool.tile([128, 256], F32, name=_nm("po"), tag="ops",
                                     bufs=2)
                    for hp in range(2):
                        pat = ppool.tile([128, 128], F16, name=_nm("pat"), tag="tps16",
                                         bufs=2)
                        nc.tensor.transpose(
                            pat[:], et_w[:, gp * HID + hp * 128:gp * HID + hp * 128 + 128],
                            ident16[:])
                        aT2 = smpool.tile([128, 128], F16, name=_nm("aT"), tag="aT",
                                          bufs=4)
                        cp(aT2[:], pat[:])
                        for par in range(2):
                            for t in range(2):
                                h_g = 2 * hp + t
                                xn = xl_nodes[0] if par == t else xl_nodes[1]
                                nc.tensor.matmul(
                                    po2[t * 64:(t + 1) * 64,
                                        hp * 128 + par * 64:hp * 128 + par * 64 + 64],
                                    xn[t * 64:t * 64 + 64,
                                       gp * HID + h_g * 64:gp * HID + h_g * 64 + 64],
                                    aT2[t * 64:t * 64 + 64, par * 64:par * 64 + 64],
                                    start=True, stop=True)
                    if bz:
                        # out_bias == 0: one ELU over both head-halves
                        e1 = smpool.tile([128, 256], F16, name=_nm("e1"), tag="e1",
                                         bufs=4)
                        nc.scalar.activation(e1[:], po2[:], AF.Exp, bias=c_zero[:])
                        r1 = smpool.tile([128, 256], F16, name=_nm("r1"), tag="r1",
                                         bufs=4)
                        nc.vector.tensor_scalar(r1[:], po2[:], 0.0, None, op0=ALU.max)
                        nc.vector.scalar_tensor_tensor(
                            outT_w[:, gp * 256:gp * 256 + 256], e1[:], 1.0, r1[:],
                            op0=ALU.min, op1=ALU.add)
                    else:
                        for hp in range(2):
                            po = po2[:, hp * 128:hp * 128 + 128]
                            ob_sl = obT[:, l * 2 + hp:l * 2 + hp + 1]
                            e1 = smpool.tile([128, 256], F16, name=_nm("e1"), tag="e1",
                                             bufs=4)
                            nc.scalar.activation(e1[:, 0:128], po, AF.Exp, bias=ob_sl)
                            r1 = smpool.tile([128, 256], F16, name=_nm("r1"), tag="r1",
                                             bufs=4)
                            nc.vector.tensor_scalar(r1[:, 0:128], po, ob_sl, 0.0,
                                                    op0=ALU.add, op1=ALU.max)
                            nc.vector.scalar_tensor_tensor(
                                outT_w[:, gp * 256 + hp * 128:gp * 256 + hp * 128 + 128],
                                e1[:, 0:128], 1.0, r1[:, 0:128],
                                op0=ALU.min, op1=ALU.add)
                    # node layout + LayerNorm + residual + mask, per gp
                    for hp in range(2):
                        pg = ppool.tile([128, 128], F16, name=_nm("pg"), tag="tps16",
                                        bufs=2)
                        nc.tensor.transpose(
                            pg[:], outT_w[:, gp * 256 + hp * 128:gp * 256 + hp * 128 + 128],
                            ident16[:])
                        cp(gn_w[:, gp * HID + hp * 128:gp * HID + hp * 128 + 128], pg[:])
                    gsl = gn_w[:, gp * HID:gp * HID + HID]
                    nc.vector.tensor_reduce(sums[:, gp:gp + 1], gsl,
                                            axis=mybir.AxisListType.X, op=ALU.add)
                    scr2 = smpool.tile([128, HID], F16, name=_nm("scr2"), tag="scr2",
                                       bufs=2)
                    nc.scalar.activation(scr2[:], gsl, AF.Square, bias=c_zero[:],
                                         accum_out=sqs[:, gp:gp + 1])
                    nc.vector.tensor_scalar(mus[:, gp:gp + 1], sums[:, gp:gp + 1],
                                            1.0 / HID, None, op0=ALU.mult)
                    nc.vector.tensor_tensor(vars_[:, gp:gp + 1], mus[:, gp:gp + 1],
                                            mus[:, gp:gp + 1], op=ALU.mult)
                    nc.vector.scalar_tensor_tensor(vars_[:, gp:gp + 1],
                                                   sqs[:, gp:gp + 1], 1.0 / HID,
                                                   vars_[:, gp:gp + 1],
                                                   op0=ALU.mult, op1=ALU.subtract)

                def rstd_batch(b):
                    # rsqrt on the vector engine: quake-style seed from the
                    # exponent bits, then two Newton steps. Keeps the scalar
                    # engine inside one activation table for the whole kernel.
                    sl4 = slice(b * 4, b * 4 + 4)
                    v4 = lnv8[:, sl4]
                    nc.vector.tensor_scalar(v4, vars_[:, sl4], LN_EPS, None,
                                            op0=ALU.add)
                    y = rstd8[:, sl4]
                    yu = y.bitcast(dt.uint32)
                    nc.vector.tensor_scalar(yu, v4.bitcast(dt.uint32), 1,
                                            0xFFFFFFFF, op0=ALU.logical_shift_right,
                                            op1=ALU.bitwise_xor)
                    nc.vector.tensor_scalar(yu, yu, 0xA0C8A620, None,
                                            op0=ALU.subtract)
                    t4 = smpool.tile([128, 4], F32, name=_nm("t4"), tag="t4", bufs=2)
                    for _ in range(2):
                        nc.vector.tensor_tensor(t4[:], y, y, op=ALU.mult)
                        nc.vector.tensor_tensor(t4[:], t4[:], v4, op=ALU.mult)
                        nc.vector.tensor_scalar(t4[:], t4[:], -0.5, 1.5,
                                                op0=ALU.mult, op1=ALU.add)
                        nc.vector.tensor_tensor(y, y, t4[:], op=ALU.mult)

                def stage_b2(gp):
                    gsl = gn_w[:, gp * HID:gp * HID + HID]
                    nc.vector.tensor_scalar(gsl, gsl, mus[:, gp:gp + 1],
                                            rstd8[:, gp:gp + 1],
                                            op0=ALU.subtract, op1=ALU.mult)
                    if not ln_id:
                        nc.vector.tensor_tensor(gsl, gsl, gam[l][:], op=ALU.mult)
                    hsl = hn_w[:, gp * HID:gp * HID + HID]
                    nc.vector.tensor_tensor(hsl, gsl, hb_w[:, gp * HID:gp * HID + HID],
                                            op=ALU.add)
                    nc.vector.tensor_scalar(hsl, hsl, mvec[:, gp:gp + 1], None,
                                            op0=ALU.mult)
                    if hT_nxt is not None:
                        for m in range(2):
                            pt2 = ppool.tile([128, 128], F32, name=_nm("pht"),
                                             tag="tpsf", bufs=1)
                            nc.tensor.transpose(
                                pt2[:],
                                hn_w[:, gp * HID + m * 128:gp * HID + m * 128 + 128],
                                ident32[:])
                            cp(hT_nxt[m][:, gp * 128:(gp + 1) * 128], pt2[:])
                    else:
                        # last layer: stream this pair's rows out now
                        for par in range(2):
                            g = gp * 2 + par
                            src = fd(hn_w[par * 64:par * 64 + 64,
                                          gp * HID:gp * HID + 1], (1, HID))
                            dst_sl = out_d[g * 64:g * 64 + 1, :]
                            dst = bass.AP(dst_sl.tensor, dst_sl.offset,
                                          [[HID, 64], [1, HID]])
                            q = nc.sync if par == 0 else nc.scalar
                            q.dma_start(dst, src)

                # software pipeline: scatters of gp complete while gp+1
                # computes; softmax of gp runs while gp-1 finishes; the
                # normalize tail runs in two rstd batches.
                for gp in range(8):
                    stage_a(gp)
                    if gp >= 1:
                        stage_a2(gp - 1)
                    if gp >= 2:
                        stage_b1(gp - 2)
                    if gp == 5:
                        rstd_batch(0)
                    if gp >= 5:
                        stage_b2(gp - 5)
                stage_a2(7)
                stage_b1(6)
                stage_b2(3)
                stage_b1(7)
                rstd_batch(1)
                for gp in range(4, 8):
                    stage_b2(gp)

                h_node_w = hn_w
                if hT_nxt is not None:
                    hT = hT_nxt

    nc.finalize()
    return nc


_CACHE = {}

def _get_nc(mh, ln_id=False, bz=False):
    key = (tuple(mh), ln_id, bz)
    if key not in _CACHE:
        _CACHE[key] = build_nc(tuple(mh), ln_id, bz)
    return _CACHE[key]


def _host_prep(x, person_mask, W_in, b_in, Wl, bl, Wr, br, att, out_bias, ln_scale, ln_bias):
    x = np.asarray(x, np.float32).reshape(BT, N, D_IN)
    m = np.asarray(person_mask).reshape(BT, N)
    W_in = np.ascontiguousarray(np.asarray(W_in, np.float16))
    b_in = np.asarray(b_in, np.float32)
    Wl = np.ascontiguousarray(np.asarray(Wl, np.float16))
    bl = np.asarray(bl, np.float32)
    Wr = np.ascontiguousarray(np.asarray(Wr, np.float16))
    br = np.asarray(br, np.float32)
    att = np.asarray(att, np.float32)
    out_bias = np.asarray(out_bias, np.float32)
    ln_scale = np.asarray(ln_scale, np.float32)
    ln_bias = np.asarray(ln_bias, np.float32)

    # ---- pack active nodes; stripe sorted graphs across cores ----
    n_g = m.sum(-1).astype(np.int64)                     # active counts
    order = np.argsort(-n_g, kind="stable")              # desc
    idxs = [np.nonzero(m[g])[0] for g in range(BT)]
    mh = []
    for s in range(G):
        n_top = n_g[order[s * NCORES]]
        mh.append(max(8, int(-(-int(n_top) // 8) * 8)))
    for k in range(0, G, 2):                             # pair-equalize
        mh[k + 1] = mh[k]
    mh = tuple(min(64, v) for v in mh)

    binT = np.zeros((128, 2), np.float32)
    for mm in range(2):
        binT[:, mm] = b_in[mm * 128:(mm + 1) * 128]
    blT = np.zeros((128, 2 * L), np.float32)
    brT = np.zeros((128, 2 * L), np.float32)
    obT = np.zeros((128, 2 * L), np.float32)
    for l in range(L):
        for mm in range(2):
            blT[:, l * 2 + mm] = bl[l, mm * 128:(mm + 1) * 128]
            brT[:, l * 2 + mm] = br[l, mm * 128:(mm + 1) * 128]
            obT[:, l * 2 + mm] = out_bias[l, mm * 128:(mm + 1) * 128]
    attBD = np.zeros((128, 2 * 2 * L), np.float32)
    for l in range(L):
        for hp in range(2):
            for t in range(2):
                attBD[t * 64:(t + 1) * 64, (l * 2 + hp) * 2 + t] = att[l, 2 * hp + t]
    att08 = (attBD * 0.8).astype(np.float16)
    att10 = attBD.astype(np.float16)
    gam_f = np.repeat(ln_scale[:, None, :], 128, 1).astype(np.float32).copy()
    bet_f = np.repeat(ln_bias[:, None, :], 128, 1).astype(np.float32).copy()
    ident16 = np.eye(128, dtype=np.float16)
    ident32 = np.eye(128, dtype=np.float32)
    swap16 = np.zeros((128, 128), np.float16)
    swap16[0:64, 64:128] = np.eye(64)
    swap16[64:128, 0:64] = np.eye(64)

    in_maps = []
    for c in range(NCORES):
        xg = np.zeros((G * 64, D_IN), np.float16)
        madd = np.full((2, 8, 64), NEG_BIG, np.float32)   # [par, gp, j]
        mvec_w = np.zeros((128, 8), np.float32)
        for s in range(G):
            gg = order[s * NCORES + c]
            n = int(n_g[gg])
            gp, par = s // 2, s % 2
            if n > 0:
                xg[s * 64:s * 64 + n] = x[gg][idxs[gg]].astype(np.float16)
                madd[par, gp, 0:n] = 0.0
                mvec_w[par * 64 + np.arange(n), gp] = 1.0
            else:
                madd[par, gp, 0] = 0.0
        maddS = np.zeros((128, 8 * 64), np.float32)
        for par in range(2):
            for hp in range(2):
                for t in range(2):
                    row = 32 * (par * 2 + hp) + t
                    maddS[row] = madd[par].reshape(-1)
        in_maps.append({
            "x_sh": xg, "w_in": W_in, "wl": Wl, "wr": Wr,
            "binT": binT, "blT": blT, "brT": brT, "obT": obT,
            "att08": att08, "att10": att10, "gam_f": gam_f, "bet_f": bet_f,
            "swap16": swap16, "maddS_w": maddS, "mvec_w": mvec_w,
            "ident16": ident16, "ident32": ident32,
        })
    return in_maps, x, m, W_in, b_in, order, idxs, n_g, mh


def kernel(**inputs) -> np.ndarray:
    in_maps, x, m, W_in, b_in, order, idxs, n_g, mh = _host_prep(**inputs)
    ln_id = (np.all(np.asarray(inputs["ln_scale"]) == 1.0)
             and np.all(np.asarray(inputs["ln_bias"]) == 0.0))
    bz = np.all(np.asarray(inputs["out_bias"]) == 0.0)
    nc = _get_nc(mh, ln_id, bz)
    res = bass_utils.run_bass_kernel_spmd(nc, in_maps, core_ids=list(range(NCORES)))
    out = np.zeros((BT, N, HID), np.float32)
    for c in range(NCORES):
        dev = res.results[c]["out"].reshape(G, 64, HID)
        for s in range(G):
            gg = order[s * NCORES + c]
            n = int(n_g[gg])
            if n > 0:
                out[gg][idxs[gg]] = dev[s, :n]
    keep = n_g > 1
    if not keep.all():
        for g in np.nonzero(~keep)[0]:
            out[g] = (x[g].astype(np.float32) @ np.asarray(W_in, np.float32)
                      + b_in)
    return out.reshape(B, T, N, HID)
